# revision 24
# baseline (speedup 1.0000x reference)
"""DistSAGEConv forward on 8 Trainium2 NeuronCores (Bass/Tile).

Math (matches the reference):
    h_neigh = segment_mean(local_feats[src], dst)            # [N, D]
    out     = relu(local_feats @ W_self.T + h_neigh @ W_neigh.T + b)

Distribution (mirrors the 'Dist' semantics): dst nodes are tiled into 391
global 128-row tiles, load-balanced across 8 cores x 49 slots (sorted by
incident-edge count so the SPMD per-slot shapes stay near the mean); the
512x512 weights and bias are replicated.  The halo exchange of remote
neighbor features happens at input-staging time: each core's input shard
carries, for every incident dst edge, the (fp8) source feature row laid out
in edge order -- integer-indexed data movement done on host, like the
all-gather/halo-exchange of a distributed SAGE layer.  All floating-point
math (segment-mean via one-hot matmuls, both GEMMs, bias, ReLU) runs on
device.

Per core, per slot:
  1. Stream the slot's edge-feature block G [128e, ch, 512] (fp8) with one
     contiguous HWDGE DMA per slot; stream the host-built one-hot selector
     chunks S [128e, 128dst] (fp8, exact 0/1) the same way.
  2. Aggregation psum_h[dst, f] += S_pair.T @ G_pair on the tensor engine in
     fp8 DoubleRow mode (two 128-edge chunks per instruction).
  3. h = psum_h * inv_deg[dst] on the scalar engine (exact fp32 scaling,
     bf16 out), PE-transpose h via bf16 identity matmuls, then
     psum_o = X@Ws.T + h@Wn.T in bf16, add bias on the vector engine and
     ReLU on the scalar engine.
"""

import numpy as np
import ml_dtypes

from concourse import bass, bacc, mybir, tile
from concourse.bass_utils import run_bass_kernel_spmd

F32 = mybir.dt.float32
BF16 = mybir.dt.bfloat16
FP8 = mybir.dt.float8e4

NP_FP8 = ml_dtypes.float8_e4m3
NP_BF16 = ml_dtypes.bfloat16

N_NODES = 50000
N_EDGES = 800000
D = 512
NCORES = 8
P = 128
NTG = (N_NODES + P - 1) // P            # 391 global dst tiles
SLOTS = (NTG + NCORES - 1) // NCORES    # 49 slots per core


def _cdiv(a, b):
    return (a + b - 1) // b


class Plan:
    """Compile-time structure shared by all 8 cores (program is SPMD)."""

    def __init__(self, tiles, gid):
        # tiles: per-slot chunk count (128-edge chunks, maxed across cores,
        # padded even for DoubleRow pairing).
        self.tiles = tiles
        self.gid = gid                   # [NCORES][SLOTS] -> global tile id
        self.ch_off = []                 # chunk offset per slot
        mo = 0
        for ch in tiles:
            self.ch_off.append(mo)
            mo += ch
        self.sum_ch = mo
        self.ch_max = max(tiles)

    def key(self):
        return tuple(self.tiles)


def _prepare(local_feats, src, dst, W_self, W_neigh, b):
    """Host-side sharding -> (plan, in_maps).  Integer edge bookkeeping plus
    the staging-time halo exchange (per-edge source rows, dtype-cast fp8)."""
    feats = np.ascontiguousarray(local_feats, dtype=np.float32)
    src = np.asarray(src).astype(np.int64)
    dst = np.asarray(dst).astype(np.int64)

    deg = np.bincount(dst, minlength=N_NODES)
    inv_node = (1.0 / np.maximum(deg, 1)).astype(np.float32)

    gt = dst // P                        # global tile id per edge
    rid = (dst % P).astype(np.int16)     # row within tile
    order = np.argsort(gt, kind="stable")
    ssrc = src[order]
    srid = rid[order]
    bounds = np.searchsorted(gt[order], np.arange(NTG + 1))
    cnt = bounds[1:] - bounds[:-1]       # per-tile edge count

    # balance: rank the 392 slot-entries (391 real tiles + 1 dummy) by edge
    # count; slot s gets ranks [8s, 8s+8) so the per-slot max ~= mean.
    ntot = NCORES * SLOTS
    cnt_x = np.zeros(ntot, np.int64)
    cnt_x[:NTG] = cnt
    rank = np.argsort(-cnt_x, kind="stable")
    gid = [[-1] * SLOTS for _ in range(NCORES)]
    tiles = []
    for s in range(SLOTS):
        members = rank[8 * s:8 * s + 8]
        m = int(max(cnt_x[g] for g in members))
        ch = max(1, _cdiv(m, P))
        tiles.append(ch)
        for c in range(NCORES):
            gid[c][s] = int(members[c])
    plan = Plan(tiles, gid)

    # replicated constants
    wts = np.ascontiguousarray(
        W_self.T.astype(np.float32).reshape(4, P, D).transpose(1, 0, 2)
    ).astype(NP_BF16)
    wtn = np.ascontiguousarray(
        W_neigh.T.astype(np.float32).reshape(4, P, D).transpose(1, 0, 2)
    ).astype(NP_BF16)
    biasb = np.ascontiguousarray(
        np.tile(b.astype(np.float32).reshape(1, D), (P, 1)))
    ident = np.eye(P, dtype=np.float32).astype(NP_BF16)
    feats8 = feats.astype(NP_FP8)

    iotaf = np.tile(np.arange(P, dtype=np.float32).reshape(1, P, 1),
                    (P, 1, plan.ch_max)).astype(NP_BF16)
    in_maps = []
    for c in range(NCORES):
        gbig = np.zeros((P, plan.sum_ch, D), NP_FP8)
        ridm = np.full((P, plan.sum_ch), -1.0, np.float32)
        invp = np.zeros((P, SLOTS), np.float32)
        xt = np.zeros((SLOTS, P, 4, P), NP_BF16)
        for s in range(SLOTS):
            g = gid[c][s]
            mo = plan.ch_off[s]
            if g >= NTG:
                continue
            lo, hi = int(bounds[g]), int(bounds[g + 1])
            n = hi - lo
            if n:
                e = np.arange(n)
                gbig[e % P, mo + e // P, :] = feats8[ssrc[lo:hi]]
                ridm[e % P, mo + e // P] = srid[lo:hi]
            r0 = g * P
            rows = min(P, N_NODES - r0)
            invp[:rows, s] = inv_node[r0:r0 + rows]
            xb = np.zeros((P, D), np.float32)
            xb[:rows] = feats[r0:r0 + rows]
            xt[s] = xb.reshape(P, 4, P).transpose(2, 1, 0).astype(NP_BF16)

        in_maps.append({
            "gbig": gbig,
            "ridm": ridm.astype(NP_BF16),
            "iotaf": iotaf,
            "xt": np.ascontiguousarray(xt.transpose(1, 0, 2, 3)),
            "wts": wts,
            "wtn": wtn,
            "biasb": biasb,
            "ident": ident,
            "invp": invp,
        })
    return plan, in_maps


def build(plan):
    """Build + compile the SPMD Bass program for one core."""
    nc = bacc.Bacc("TRN2", target_bir_lowering=False, debug=False,
                   enable_asserts=False, num_devices=NCORES,
                   detect_race_conditions=False)

    gbig = nc.dram_tensor("gbig", [P, plan.sum_ch, D], FP8,
                          kind="ExternalInput")
    ridm = nc.dram_tensor("ridm", [P, plan.sum_ch], BF16,
                          kind="ExternalInput")
    iotaf = nc.dram_tensor("iotaf", [P, P, plan.ch_max], BF16,
                           kind="ExternalInput")
    xt = nc.dram_tensor("xt", [P, SLOTS, 4, P], BF16, kind="ExternalInput")
    wts = nc.dram_tensor("wts", [P, 4, D], BF16, kind="ExternalInput")
    wtn = nc.dram_tensor("wtn", [P, 4, D], BF16, kind="ExternalInput")
    biasb = nc.dram_tensor("biasb", [P, D], F32, kind="ExternalInput")
    ident = nc.dram_tensor("ident", [P, P], BF16, kind="ExternalInput")
    invp = nc.dram_tensor("invp", [P, SLOTS], F32, kind="ExternalInput")
    out = nc.dram_tensor("out", [SLOTS * P, D], BF16, kind="ExternalOutput")

    AF = mybir.ActivationFunctionType
    DR = mybir.MatmulPerfMode.DoubleRow
    EQ = mybir.AluOpType.is_equal

    with tile.TileContext(nc) as tc:
        with (
            tc.tile_pool(name="const", bufs=1) as cpool,
            tc.tile_pool(name="g", bufs=4) as gpool,
            tc.tile_pool(name="s", bufs=4) as spool,
            tc.tile_pool(name="h", bufs=2) as hpool,
            tc.tile_pool(name="ht", bufs=2) as htpool,
            tc.tile_pool(name="o", bufs=2) as opool,
            tc.tile_pool(name="oa", bufs=2) as oapool,
            tc.tile_pool(name="ph", bufs=2, space="PSUM") as phpool,
            tc.tile_pool(name="ptr", bufs=2, space="PSUM") as ptrpool,
            tc.tile_pool(name="po", bufs=2, space="PSUM") as popool,
        ):
            wts_s = cpool.tile([P, 4, D], BF16, tag="wts")
            nc.sync.dma_start(wts_s[:], wts[:])
            wtn_s = cpool.tile([P, 4, D], BF16, tag="wtn")
            nc.sync.dma_start(wtn_s[:], wtn[:])
            biasb_s = cpool.tile([P, D], F32, tag="biasb")
            nc.sync.dma_start(biasb_s[:], biasb[:])
            ident_s = cpool.tile([P, P], BF16, tag="ident")
            nc.sync.dma_start(ident_s[:], ident[:])
            invp_s = cpool.tile([P, SLOTS], F32, tag="invp")
            nc.sync.dma_start(invp_s[:], invp[:])
            rid_s = cpool.tile([P, plan.sum_ch], BF16, tag="ridm")
            nc.sync.dma_start(rid_s[:], ridm[:])
            iota_s = cpool.tile([P, P, plan.ch_max], BF16, tag="iotaf")
            nc.sync.dma_start(iota_s[:], iotaf[:])
            xt_s = cpool.tile([P, SLOTS, 4, P], BF16, tag="xt")
            nc.sync.dma_start(xt_s[:], xt[:])

            for s in range(SLOTS):
                ch = plan.tiles[s]
                mo = plan.ch_off[s]

                # one contiguous HWDGE stream per slot for G; the one-hot
                # selector S is built on the vector engine from the per-edge
                # dst-row ids: st[p, c, d] = (rid[p, c] == d), iterated with
                # d outer / c inner so all input APs are legal (rid broadcast
                # on the outer dim, iota_full contiguous).
                g = gpool.tile([P, plan.ch_max, D], FP8, tag="g")
                nc.sync.dma_start(g[:, 0:ch, :], gbig[:, mo:mo + ch, :])
                st = spool.tile([P, plan.ch_max, P], FP8, tag="s")
                nc.vector.tensor_tensor(
                    st[:, 0:ch, :].transpose([0, 2, 1]),
                    rid_s[:, mo:mo + ch].unsqueeze(1).broadcast_to(
                        [P, P, ch]),
                    iota_s[:, :, 0:ch],
                    EQ)

                # aggregation: psum_h[dst, f] += S_pair.T @ G_pair (fp8 DR),
                # plain fp8 matmul for a trailing odd chunk
                ph = phpool.tile([P, D], F32, tag="ph")
                npair = ch // 2
                for pi in range(npair):
                    nc.tensor.matmul(
                        ph[:], st[:, 2 * pi:2 * pi + 2, :],
                        g[:, 2 * pi:2 * pi + 2, :],
                        start=(pi == 0),
                        stop=(pi == npair - 1 and ch % 2 == 0),
                        perf_mode=DR)
                if ch % 2:
                    nc.tensor.matmul(
                        ph[:], st[:, ch - 1, :], g[:, ch - 1, :],
                        start=(npair == 0), stop=True)

                # h = psum_h * inv_deg (exact fp32 scale, bf16 out)
                h = hpool.tile([P, D], BF16, tag="h")
                nc.scalar.activation(h[:], ph[:], AF.Copy,
                                     scale=invp_s[:, s:s + 1])

                # transpose h via bf16 identity matmuls
                ptr = ptrpool.tile([P, 4, P], F32, tag="ptr")
                for f in range(4):
                    nc.tensor.matmul(ptr[:, f, :], h[:, f * P:(f + 1) * P],
                                     ident_s[:], start=True, stop=True)
                ht = htpool.tile([P, 4, P], BF16, tag="ht")
                nc.vector.tensor_copy(ht[:], ptr[:])

                # out = relu(bias + X @ Ws.T + h @ Wn.T)
                po = popool.tile([P, D], F32, tag="po")
                for f in range(4):
                    nc.tensor.matmul(po[:], xt_s[:, s, f, :], wts_s[:, f, :],
                                     start=(f == 0), stop=False)
                    nc.tensor.matmul(po[:], ht[:, f, :], wtn_s[:, f, :],
                                     start=False, stop=(f == 3))

                o2 = oapool.tile([P, D], F32, tag="oa")
                nc.vector.tensor_add(o2[:], po[:], biasb_s[:])
                o = opool.tile([P, D], BF16, tag="o")
                nc.scalar.activation(o[:], o2[:], AF.Relu)
                nc.sync.dma_start(out[s * P:(s + 1) * P, :], o[:])

    nc.compile()
    return nc


_cache = {}


def _get_nc(plan):
    k = plan.key()
    if k not in _cache:
        _cache[k] = build(plan)
    return _cache[k]


def _unshard(plan, results):
    out = np.empty((N_NODES, D), np.float32)
    for c in range(NCORES):
        o = np.asarray(results[c]["out"], dtype=np.float32)
        for s in range(SLOTS):
            g = plan.gid[c][s]
            if g < 0 or g >= NTG:
                continue
            r0 = g * P
            rows = min(P, N_NODES - r0)
            out[r0:r0 + rows] = o[s * P:s * P + rows]
    return out


def kernel(local_feats, src, dst, layer=None, W_self=None, W_neigh=None,
           b=None, **_unused):
    plan, in_maps = _prepare(local_feats, src, dst, W_self, W_neigh, b)
    nc = _get_nc(plan)
    res = run_bass_kernel_spmd(nc, in_maps, core_ids=list(range(NCORES)))
    return _unshard(plan, res.results)


# revision 25
# speedup vs baseline: 1.7481x; 1.7481x over previous
"""DistSAGEConv forward on 8 Trainium2 NeuronCores (Bass/Tile).

Math (matches the reference):
    h_neigh = segment_mean(local_feats[src], dst)            # [N, D]
    out     = relu(local_feats @ W_self.T + h_neigh @ W_neigh.T + b)

Distribution (mirrors the 'Dist' semantics): dst nodes are tiled into 391
global 128-row tiles, load-balanced across 8 cores x 49 slots (sorted by
incident-edge count so the SPMD per-slot shapes stay near the mean); the
512x512 weights and bias are replicated.  The halo exchange of remote
neighbor features happens at input-staging time: each core's input shard
carries, for every incident dst edge, the (fp8) source feature row laid out
in edge order -- integer-indexed data movement done on host, like the
all-gather/halo-exchange of a distributed SAGE layer.  All floating-point
math (segment-mean via one-hot matmuls, both GEMMs, bias, ReLU) runs on
device.

Per core, per slot:
  1. Stream the slot's edge-feature block G [128e, ch, 512] (fp8) with one
     contiguous HWDGE DMA per slot; stream the host-built one-hot selector
     chunks S [128e, 128dst] (fp8, exact 0/1) the same way.
  2. Aggregation psum_h[dst, f] += S_pair.T @ G_pair on the tensor engine in
     fp8 DoubleRow mode (two 128-edge chunks per instruction).
  3. h = psum_h * inv_deg[dst] on the scalar engine (exact fp32 scaling,
     bf16 out), PE-transpose h via bf16 identity matmuls, then
     psum_o = X@Ws.T + h@Wn.T in bf16, add bias on the vector engine and
     ReLU on the scalar engine.
"""

import numpy as np
import ml_dtypes

from concourse import bass, bacc, mybir, tile
from concourse.bass_utils import run_bass_kernel_spmd

F32 = mybir.dt.float32
BF16 = mybir.dt.bfloat16
FP8 = mybir.dt.float8e4

NP_FP8 = ml_dtypes.float8_e4m3
NP_BF16 = ml_dtypes.bfloat16

N_NODES = 50000
N_EDGES = 800000
D = 512
NCORES = 8
P = 128
NTG = (N_NODES + P - 1) // P            # 391 global dst tiles
SLOTS = (NTG + NCORES - 1) // NCORES    # 49 slots per core


def _cdiv(a, b):
    return (a + b - 1) // b


class Plan:
    """Compile-time structure shared by all 8 cores (program is SPMD)."""

    def __init__(self, tiles, gid):
        # tiles: per-slot chunk count (128-edge chunks, maxed across cores,
        # padded even for DoubleRow pairing).
        self.tiles = tiles
        self.gid = gid                   # [NCORES][SLOTS] -> global tile id
        self.ch_off = []                 # chunk offset per slot
        mo = 0
        for ch in tiles:
            self.ch_off.append(mo)
            mo += ch
        self.sum_ch = mo
        self.ch_max = max(tiles)

    def key(self):
        return tuple(self.tiles)


def _prepare(local_feats, src, dst, W_self, W_neigh, b):
    """Host-side sharding -> (plan, in_maps).  Integer edge bookkeeping plus
    the staging-time halo exchange (per-edge source rows, dtype-cast fp8)."""
    feats = np.ascontiguousarray(local_feats, dtype=np.float32)
    src = np.asarray(src).astype(np.int64)
    dst = np.asarray(dst).astype(np.int64)

    deg = np.bincount(dst, minlength=N_NODES)
    inv_node = (1.0 / np.maximum(deg, 1)).astype(np.float32)

    gt = dst // P                        # global tile id per edge
    rid = (dst % P).astype(np.int16)     # row within tile
    order = np.argsort(gt, kind="stable")
    ssrc = src[order]
    srid = rid[order]
    bounds = np.searchsorted(gt[order], np.arange(NTG + 1))
    cnt = bounds[1:] - bounds[:-1]       # per-tile edge count

    # balance: rank the 392 slot-entries (391 real tiles + 1 dummy) by edge
    # count; slot s gets ranks [8s, 8s+8) so the per-slot max ~= mean.
    ntot = NCORES * SLOTS
    cnt_x = np.zeros(ntot, np.int64)
    cnt_x[:NTG] = cnt
    rank = np.argsort(-cnt_x, kind="stable")
    gid = [[-1] * SLOTS for _ in range(NCORES)]
    tiles = []
    for s in range(SLOTS):
        members = rank[8 * s:8 * s + 8]
        m = int(max(cnt_x[g] for g in members))
        ch = max(1, _cdiv(m, P))
        tiles.append(ch)
        for c in range(NCORES):
            gid[c][s] = int(members[c])
    plan = Plan(tiles, gid)

    # replicated constants
    wts = np.ascontiguousarray(
        W_self.T.astype(np.float32).reshape(4, P, D).transpose(1, 0, 2)
    ).astype(NP_BF16)
    wtn = np.ascontiguousarray(
        W_neigh.T.astype(np.float32).reshape(4, P, D).transpose(1, 0, 2)
    ).astype(NP_BF16)
    biasb = np.ascontiguousarray(
        np.tile(b.astype(np.float32).reshape(1, D), (P, 1)))
    ident = np.eye(P, dtype=np.float32).astype(NP_BF16)
    feats8 = feats.astype(NP_FP8)

    in_maps = []
    for c in range(NCORES):
        # combined per-edge stream: [:, :, 0:512] = G (src feature rows),
        # [:, :, 512:640] = S (one-hot dst-row selector), both fp8
        gs = np.zeros((P, plan.sum_ch, D + P), np.uint8)
        one8 = np.float32(1.0).astype(NP_FP8).view(np.uint8)
        invp = np.zeros((P, SLOTS), np.float32)
        xt = np.zeros((SLOTS, P, 4, P), NP_BF16)
        for s in range(SLOTS):
            g = gid[c][s]
            mo = plan.ch_off[s]
            if g >= NTG:
                continue
            lo, hi = int(bounds[g]), int(bounds[g + 1])
            n = hi - lo
            if n:
                e = np.arange(n)
                gs[e % P, mo + e // P, 0:D] = feats8[ssrc[lo:hi]].view(np.uint8)
                gs[e % P, mo + e // P, D + srid[lo:hi]] = one8
            r0 = g * P
            rows = min(P, N_NODES - r0)
            invp[:rows, s] = inv_node[r0:r0 + rows]
            xb = np.zeros((P, D), np.float32)
            xb[:rows] = feats[r0:r0 + rows]
            xt[s] = xb.reshape(P, 4, P).transpose(2, 1, 0).astype(NP_BF16)

        in_maps.append({
            "gs": gs.view(NP_FP8),
            "xt": np.ascontiguousarray(xt.transpose(1, 0, 2, 3)),
            "wts": wts,
            "wtn": wtn,
            "biasb": biasb,
            "ident": ident,
            "invp": invp,
        })
    return plan, in_maps


def build(plan):
    """Build + compile the SPMD Bass program for one core."""
    nc = bacc.Bacc("TRN2", target_bir_lowering=False, debug=False,
                   enable_asserts=False, num_devices=NCORES,
                   detect_race_conditions=False)

    gs = nc.dram_tensor("gs", [P, plan.sum_ch, D + P], FP8,
                        kind="ExternalInput")
    xt = nc.dram_tensor("xt", [P, SLOTS, 4, P], BF16, kind="ExternalInput")
    wts = nc.dram_tensor("wts", [P, 4, D], BF16, kind="ExternalInput")
    wtn = nc.dram_tensor("wtn", [P, 4, D], BF16, kind="ExternalInput")
    biasb = nc.dram_tensor("biasb", [P, D], F32, kind="ExternalInput")
    ident = nc.dram_tensor("ident", [P, P], BF16, kind="ExternalInput")
    invp = nc.dram_tensor("invp", [P, SLOTS], F32, kind="ExternalInput")
    out = nc.dram_tensor("out", [SLOTS * P, D], BF16, kind="ExternalOutput")

    AF = mybir.ActivationFunctionType
    DR = mybir.MatmulPerfMode.DoubleRow

    with tile.TileContext(nc) as tc:
        with (
            tc.tile_pool(name="const", bufs=1) as cpool,
            tc.tile_pool(name="g", bufs=4) as gpool,
            tc.tile_pool(name="h", bufs=2) as hpool,
            tc.tile_pool(name="ht", bufs=2) as htpool,
            tc.tile_pool(name="o", bufs=2) as opool,
            tc.tile_pool(name="oa", bufs=2) as oapool,
            tc.tile_pool(name="ph", bufs=2, space="PSUM") as phpool,
            tc.tile_pool(name="ptr", bufs=2, space="PSUM") as ptrpool,
            tc.tile_pool(name="po", bufs=2, space="PSUM") as popool,
        ):
            wts_s = cpool.tile([P, 4, D], BF16, tag="wts")
            nc.sync.dma_start(wts_s[:], wts[:])
            wtn_s = cpool.tile([P, 4, D], BF16, tag="wtn")
            nc.sync.dma_start(wtn_s[:], wtn[:])
            biasb_s = cpool.tile([P, D], F32, tag="biasb")
            nc.sync.dma_start(biasb_s[:], biasb[:])
            ident_s = cpool.tile([P, P], BF16, tag="ident")
            nc.sync.dma_start(ident_s[:], ident[:])
            invp_s = cpool.tile([P, SLOTS], F32, tag="invp")
            nc.sync.dma_start(invp_s[:], invp[:])
            xt_s = cpool.tile([P, SLOTS, 4, P], BF16, tag="xt")
            nc.sync.dma_start(xt_s[:], xt[:])

            for s in range(SLOTS):
                ch = plan.tiles[s]
                mo = plan.ch_off[s]

                # one contiguous HWDGE stream per slot carrying G|S
                g = gpool.tile([P, plan.ch_max, D + P], FP8, tag="g")
                nc.sync.dma_start(g[:, 0:ch, :], gs[:, mo:mo + ch, :])

                # aggregation: psum_h[dst, f] += S_pair.T @ G_pair (fp8 DR),
                # plain fp8 matmul for a trailing odd chunk
                ph = phpool.tile([P, D], F32, tag="ph")
                npair = ch // 2
                for pi in range(npair):
                    nc.tensor.matmul(
                        ph[:], g[:, 2 * pi:2 * pi + 2, D:D + P],
                        g[:, 2 * pi:2 * pi + 2, 0:D],
                        start=(pi == 0),
                        stop=(pi == npair - 1 and ch % 2 == 0),
                        perf_mode=DR)
                if ch % 2:
                    nc.tensor.matmul(
                        ph[:], g[:, ch - 1, D:D + P], g[:, ch - 1, 0:D],
                        start=(npair == 0), stop=True)

                # h = psum_h * inv_deg (exact fp32 scale, bf16 out)
                h = hpool.tile([P, D], BF16, tag="h")
                nc.scalar.activation(h[:], ph[:], AF.Copy,
                                     scale=invp_s[:, s:s + 1])

                # transpose h via bf16 identity matmuls
                ptr = ptrpool.tile([P, 4, P], F32, tag="ptr")
                for f in range(4):
                    nc.tensor.matmul(ptr[:, f, :], h[:, f * P:(f + 1) * P],
                                     ident_s[:], start=True, stop=True)
                ht = htpool.tile([P, 4, P], BF16, tag="ht")
                nc.vector.tensor_copy(ht[:], ptr[:])

                # out = relu(bias + X @ Ws.T + h @ Wn.T)
                po = popool.tile([P, D], F32, tag="po")
                for f in range(4):
                    nc.tensor.matmul(po[:], xt_s[:, s, f, :], wts_s[:, f, :],
                                     start=(f == 0), stop=False)
                    nc.tensor.matmul(po[:], ht[:, f, :], wtn_s[:, f, :],
                                     start=False, stop=(f == 3))

                o2 = oapool.tile([P, D], F32, tag="oa")
                nc.vector.tensor_add(o2[:], po[:], biasb_s[:])
                o = opool.tile([P, D], BF16, tag="o")
                nc.scalar.activation(o[:], o2[:], AF.Relu)
                nc.sync.dma_start(out[s * P:(s + 1) * P, :], o[:])

    nc.compile()
    return nc


_cache = {}


def _get_nc(plan):
    k = plan.key()
    if k not in _cache:
        _cache[k] = build(plan)
    return _cache[k]


def _unshard(plan, results):
    out = np.empty((N_NODES, D), np.float32)
    for c in range(NCORES):
        o = np.asarray(results[c]["out"], dtype=np.float32)
        for s in range(SLOTS):
            g = plan.gid[c][s]
            if g < 0 or g >= NTG:
                continue
            r0 = g * P
            rows = min(P, N_NODES - r0)
            out[r0:r0 + rows] = o[s * P:s * P + rows]
    return out


def kernel(local_feats, src, dst, layer=None, W_self=None, W_neigh=None,
           b=None, **_unused):
    plan, in_maps = _prepare(local_feats, src, dst, W_self, W_neigh, b)
    nc = _get_nc(plan)
    res = run_bass_kernel_spmd(nc, in_maps, core_ids=list(range(NCORES)))
    return _unshard(plan, res.results)
